# revision 21
# baseline (speedup 1.0000x reference)
"""nn_BlockCirculantLinear on 8 Trainium2 cores (Bass/Tile, bf16, packed mid).

Math.  Per output block o (8 blocks of P=512):
    y_o = sum_i real(IFFT(Lam[o,i] * FFT(x_i * sf_i)))
factors into three real linear stages (real-DFT coordinates, frequency
pair (f, P-f) folded into a 2x2 real block):
  1. forward  : X_i = Fe @ (sf*x)_i^T      -- dense 512x512 per block i
  2. middle   : Y_o = sum_i M_oi X_i       -- per-frequency-pair 2x2 mixes
  3. inverse  : y_o^T = Fi @ Y_o           -- dense 512x512 per block o

The middle stage is 2x2-block-diagonal (1.5% dense), so instead of dense
128x128 tiles over (o,i) [256 matmuls/chunk] we re-pack partitions so one
matmul covers 8 frequency pairs for ALL (i -> o) at once [32 matmuls/chunk]:
  packed tile T=(ct,g): partition p = i*16 + q  holds coord ct*128+g*16+q
  of block i; lhsT[T][i*16+q_in, o*16+q_out] = M[o,i,C+q_out,C+q_in].
The repack is a partition-slab DMA shuffle.  To keep every shuffle DMA a
single-partition-dim 3-dim AP, forward outputs are produced PRE-PERMUTED
(per 128-coord tile, PSUM partition p holds coord (p%8)*16 + p//8, folded
into Fe's row order); the inverse's K-dim uses the same permuted order
(folded into Fi's column order).  Zero-cost on device.

Per chunk of 512 rows: 128 fwd + 32 mid + 128 inv matmuls of
[K=128,M=128,N=512] bf16 (213ns each) = 61us PE time; DMA: x-in 8 (SP),
Xshuf 32 (split Pool-SWDGE/Act), Yshuf 32 (split Act/Pool), y-out 8 (SP)
= 80 DMAs, 16.8MB.  Stages are software-pipelined across chunks (mid and
inv of chunk c-1 interleave between the two fwd halves of chunk c) so
shuffle-DMA tails complete under fwd compute; fwd handles input blocks in
pairs sharing each lhsT back-to-back (halves LD_WEIGHTS on hw); PSUM
pools 4/2/2 banks; evictions alternate DVE/ACT explicitly.

Sharding: data-parallel -- 16384 rows split 8 ways; constants replicated.
sign_flip folded into x on host; bias added on host after gathering.
"""
import os
from contextlib import ExitStack

import numpy as np
import ml_dtypes

import concourse.mybir as mybir
import concourse.bacc as bacc
import concourse.tile as tile
from concourse.bass_utils import run_bass_kernel_spmd

N_CORES = 8
ROWS = 16384
RPC = ROWS // N_CORES      # 2048 rows per core
F = 4096
P = 512
NBLK = 8
CHUNK = 512                # rows per pipelined chunk (= matmul free dim)
NCT = 4                    # 128-coord tiles per block
_NC_CACHE = {}

DT = mybir.dt.bfloat16
DTO = mybir.dt.float32

# partition p of a forward-output tile holds local coord PERM[p]
PERM = np.array([(p % 8) * 16 + p // 8 for p in range(128)])


def build_transforms(spectral_real, spectral_imag, dtype=np.float64):
    """Fe [c, feat], Fi [t, c], M [o, i, c_out, c_in] (2x2 block diagonal)."""
    s = np.arange(P)
    f = np.arange(1, P // 2)
    theta = 2 * np.pi * np.outer(f, s) / P

    Fe = np.zeros((P, P), dtype)
    Fe[0, :] = 1.0
    Fe[1, :] = (-1.0) ** s
    Fe[2::2, :] = np.cos(theta)
    Fe[3::2, :] = -np.sin(theta)

    Fi = np.zeros((P, P), dtype)
    Fi[:, 0] = 1.0 / P
    Fi[:, 1] = ((-1.0) ** s) / P
    Fi[:, 2::2] = np.cos(theta).T / P
    Fi[:, 3::2] = -np.sin(theta).T / P

    lam_r = spectral_real.astype(dtype)
    lam_i = spectral_imag.astype(dtype)
    M = np.zeros((NBLK, NBLK, P, P), dtype)
    M[:, :, 0, 0] = lam_r[:, :, 0]
    M[:, :, 1, 1] = lam_r[:, :, P // 2]
    l1r = lam_r[:, :, 1:P // 2]; l1i = lam_i[:, :, 1:P // 2]
    l2r = lam_r[:, :, :P // 2:-1]; l2i = lam_i[:, :, :P // 2:-1]
    ce = np.arange(2, P, 2); co = ce + 1
    M[:, :, ce, ce] = l1r + l2r
    M[:, :, ce, co] = l2i - l1i
    M[:, :, co, ce] = l1i - l2i
    M[:, :, co, co] = l1r + l2r
    return Fe, Fi, M


def host_transforms(spectral_real, spectral_imag):
    Fe, Fi, M = build_transforms(spectral_real, spectral_imag)
    # fwd lhsT [k, (kc*4+mt)*128 + p] = Fe[mt*128 + PERM[p], kc*128 + k]
    fwdT = np.zeros((128, 16 * 128), np.float32)
    for kc in range(4):
        for mt in range(4):
            blk = Fe[mt * 128:(mt + 1) * 128, kc * 128:(kc + 1) * 128]
            fwdT[:, (kc * 4 + mt) * 128:(kc * 4 + mt + 1) * 128] = \
                blk[PERM, :].T
    # inv lhsT [p, (ct*4+tt)*128 + t] = Fi[tt*128 + t, ct*128 + PERM[p]]
    invT = np.zeros((128, 16 * 128), np.float32)
    for ct in range(4):
        for tt in range(4):
            blk = Fi[tt * 128:(tt + 1) * 128, ct * 128:(ct + 1) * 128]
            invT[:, (ct * 4 + tt) * 128:(ct * 4 + tt + 1) * 128] = \
                blk[:, PERM].T
    # mid lhsT [i*16+q_in, T*128 + o*16+q_out] = M[o,i,C+q_out,C+q_in]
    midT = np.zeros((128, 32 * 128), np.float32)
    for ct in range(4):
        for g in range(8):
            T = ct * 8 + g
            C = ct * 128 + g * 16
            for o in range(NBLK):
                for i in range(NBLK):
                    midT[i * 16:(i + 1) * 16,
                         T * 128 + o * 16:T * 128 + (o + 1) * 16] = \
                        M[o, i, C:C + 16, C:C + 16].T
    bf = ml_dtypes.bfloat16
    return fwdT.astype(bf), invT.astype(bf), midT.astype(bf)


def build_nc(repeat: int = 1):
    key = (CHUNK, repeat)
    if key in _NC_CACHE:
        return _NC_CACHE[key]
    nc = bacc.Bacc("TRN2", target_bir_lowering=False, debug=False,
                   num_devices=N_CORES)
    xT = nc.dram_tensor("xT", [F, RPC], DT, kind="ExternalInput")
    fwdT = nc.dram_tensor("fwdT", [128, 16 * 128], DT, kind="ExternalInput")
    invT = nc.dram_tensor("invT", [128, 16 * 128], DT, kind="ExternalInput")
    midT = nc.dram_tensor("midT", [128, 32 * 128], DT, kind="ExternalInput")
    yT = nc.dram_tensor("yT", [F, RPC], DT, kind="ExternalOutput")

    n_chunks = RPC // CHUNK

    with tile.TileContext(nc) as tc:
        with ExitStack() as ctx:
            const = ctx.enter_context(tc.tile_pool(name="const", bufs=1))
            fwd_sb = const.tile([128, 16 * 128], DT)
            inv_sb = const.tile([128, 16 * 128], DT)
            mid_sb = const.tile([128, 32 * 128], DT)
            nc.sync.dma_start(fwd_sb[:], fwdT[:])
            nc.sync.dma_start(inv_sb[:], invT[:])
            nc.sync.dma_start(mid_sb[:], midT[:])

            xpool = ctx.enter_context(tc.tile_pool(name="x", bufs=8))
            Xpool = ctx.enter_context(tc.tile_pool(name="X", bufs=48))
            pkpool = ctx.enter_context(tc.tile_pool(name="pk", bufs=3))
            pypool = ctx.enter_context(tc.tile_pool(name="py", bufs=3))
            Ypool = ctx.enter_context(tc.tile_pool(name="Y", bufs=40))
            opool = ctx.enter_context(tc.tile_pool(name="out", bufs=4))
            psf = ctx.enter_context(tc.tile_pool(name="psf", bufs=4,
                                                 space="PSUM"))
            psm = ctx.enter_context(tc.tile_pool(name="psm", bufs=2,
                                                 space="PSUM"))
            psi = ctx.enter_context(tc.tile_pool(name="psi", bufs=2,
                                                 space="PSUM"))

            ev_state = [0]

            def evict(out, in_):
                # alternate DVE/ACT explicitly; the auto-scheduler unbalances
                if ev_state[0] % 2 == 0:
                    nc.vector.tensor_copy(out=out, in_=in_)
                else:
                    nc.scalar.copy(out=out, in_=in_)
                ev_state[0] += 1

            def emit_loads(c):
                r0 = c * CHUNK
                xb = {}
                for i in range(NBLK):
                    t = xpool.tile([128, 4 * CHUNK], DT, tag="x", name="xt")
                    nc.sync.dma_start(
                        t.rearrange("p (kc col) -> p kc col", kc=4),
                        xT[i * 512:(i + 1) * 512, r0:r0 + CHUNK].rearrange(
                            "(kc p) col -> p kc col", kc=4))
                    xb[i] = t
                return xb

            def emit_fwd_half(xb, pk, i_lo, i_hi):
                """fwd matmuls + evict + Xshuf for blocks [i_lo, i_hi).

                i handled in pairs sharing each lhsT tile back-to-back so
                the PE reuses loaded weights (halves LD_WEIGHTS on hw).
                """
                for i0 in range(i_lo, i_hi, 2):
                    for mt in range(NCT):
                        ps = {ii: psf.tile([128, CHUNK], DTO, tag="f",
                                           name="fps") for ii in (0, 1)}
                        for kc in range(4):
                            j = (kc * 4 + mt) * 128
                            for ii in (0, 1):
                                nc.tensor.matmul(
                                    ps[ii][:], fwd_sb[:, j:j + 128],
                                    xb[i0 + ii][:,
                                                kc * CHUNK:(kc + 1) * CHUNK],
                                    start=(kc == 0), stop=(kc == 3))
                        for ii in (0, 1):
                            i = i0 + ii
                            t = Xpool.tile([128, CHUNK], DT, tag="X",
                                           name="Xt")
                            evict(t[:], ps[ii][:])
                            eng = nc.gpsimd if i % 2 == 0 else nc.scalar
                            eng.dma_start(
                                pk[mt][i * 16:(i + 1) * 16, :].rearrange(
                                    "q (g col) -> q g col", g=8),
                                t[:])

            def emit_mid(pk):
                """packed mid matmuls + evict + Yshuf; returns Y tiles."""
                Yt = {}
                for ct in range(NCT):
                    py = pypool.tile([128, 8 * CHUNK], DT, tag="py",
                                     name="pyt")
                    for g in range(8):
                        T = ct * 8 + g
                        ps = psm.tile([128, CHUNK], DTO, tag="m", name="mps")
                        nc.tensor.matmul(
                            ps[:], mid_sb[:, T * 128:(T + 1) * 128],
                            pk[ct][:, g * CHUNK:(g + 1) * CHUNK],
                            start=True, stop=True)
                        evict(py[:, g * CHUNK:(g + 1) * CHUNK], ps[:])
                    for o in range(NBLK):
                        t = Ypool.tile([128, CHUNK], DT, tag="Y", name="Yt")
                        eng = nc.scalar if o % 2 == 0 else nc.gpsimd
                        eng.dma_start(
                            t[:],
                            py[o * 16:(o + 1) * 16, :].rearrange(
                                "q (g col) -> q g col", g=8))
                        Yt[o, ct] = t
                return Yt

            def emit_inv(Yt, c):
                r0 = c * CHUNK
                for o in range(NBLK):
                    ob = opool.tile([128, 4 * CHUNK], DT, tag="o", name="ot")
                    for tt in range(4):
                        ps = psi.tile([128, CHUNK], DTO, tag="i", name="ips")
                        for ct in range(4):
                            j = (ct * 4 + tt) * 128
                            nc.tensor.matmul(
                                ps[:], inv_sb[:, j:j + 128], Yt[o, ct][:],
                                start=(ct == 0), stop=(ct == 3))
                        evict(ob[:, tt * CHUNK:(tt + 1) * CHUNK], ps[:])
                    nc.sync.dma_start(
                        yT[o * 512:(o + 1) * 512, r0:r0 + CHUNK].rearrange(
                            "(tt p) col -> p tt col", tt=4),
                        ob.rearrange("p (tt col) -> p tt col", tt=4))

            def body(_=None):
                # software pipeline on the PE queue:
                #   loads(c) fwdA(c) mid(c-1) fwdB(c) inv(c-1)
                # so shuffle-DMA tails of chunk c-1 complete under fwd(c).
                prev_pk = None
                prev_c = None
                for c in range(n_chunks):
                    xb = emit_loads(c)
                    pk = {ct: pkpool.tile([128, 8 * CHUNK], DT, tag="pk",
                                          name="pkt") for ct in range(NCT)}
                    emit_fwd_half(xb, pk, 0, NBLK // 2)
                    if prev_pk is not None:
                        Yt = emit_mid(prev_pk)
                    emit_fwd_half(xb, pk, NBLK // 2, NBLK)
                    if prev_pk is not None:
                        emit_inv(Yt, prev_c)
                    prev_pk, prev_c = pk, c
                Yt = emit_mid(prev_pk)
                emit_inv(Yt, prev_c)

            if repeat == 1:
                body()
            else:
                with tc.For_i(0, repeat, 1) as it:
                    body(it)
    nc.compile()
    _NC_CACHE[key] = nc
    return nc


def make_in_maps(x, spectral_real, spectral_imag, sign_flip):
    fwdT, invT, midT = host_transforms(spectral_real, spectral_imag)
    bf = ml_dtypes.bfloat16
    xs = (x.reshape(-1, F) * sign_flip[None, :].astype(np.float32))
    in_maps = []
    for c in range(N_CORES):
        shard = xs[c * RPC:(c + 1) * RPC]
        in_maps.append({
            "xT": np.ascontiguousarray(shard.T).astype(bf),
            "fwdT": fwdT, "invT": invT, "midT": midT,
        })
    return in_maps


def kernel(x, spectral_real, spectral_imag, sign_flip, bias):
    x = np.asarray(x, np.float32)
    spectral_real = np.asarray(spectral_real, np.float32)
    spectral_imag = np.asarray(spectral_imag, np.float32)
    sign_flip = np.asarray(sign_flip, np.float32)
    bias = np.asarray(bias, np.float32)
    batch_shape = x.shape[:-1]

    in_maps = make_in_maps(x, spectral_real, spectral_imag, sign_flip)
    nc = build_nc()
    res = run_bass_kernel_spmd(nc, in_maps, list(range(N_CORES)))
    y = np.concatenate(
        [np.ascontiguousarray(
            np.asarray(res.results[c]["yT"], np.float32).T)
         for c in range(N_CORES)],
        axis=0)
    y = y + bias[None, :]
    return y.reshape(*batch_shape, F).astype(np.float32)
